# revision 1
# baseline (speedup 1.0000x reference)
"""Causal multi-head self-attention with RoPE on 8 Trainium2 NeuronCores.

Sharding: batch (4) x query-half (2) -> 8 cores, no collectives.
Each core computes full K/V for its batch; query rows are split between the
two cores of a batch in a causally-balanced schedule (4 slots of 256 rows
with 16/12/8/4 key-blocks each), so both halves do equal attention work
under one shared SPMD program.  Causal masking is multiplicative mask input
data, so the same program serves both halves.

Everything is computed in transposed [feature, seq] layout so no on-device
transposes are needed:
  K^T/Q^T = W^T.T @ X^T          (per 128-row head pair)
  RoPE    = cos*x + sin*(P@x)    (P = constant pair-rotation matrix, one
                                  small extra matmul per tile)
  S^T     = Krot^T.T-slice @ Qrot^T  (keys on partitions -> softmax runs
                                      along the partition axis)
  exp     = ACT Exp(scale=1/8) -> bf16
  A^T,l   = [V|1].T-free matmul accumulated over key blocks in PSUM
  out     = A^T.T @ Wo^T         (natural [seq, feature] output layout)

Matmuls use fp32r (full PE rate at N>=256, ~FP22 precision) except the
attention-value matmul which is bf16 (exp output x V).
"""

import os
import sys
import math

if "/opt/trn_rl_repo" not in sys.path:
    sys.path.append("/opt/trn_rl_repo")

import numpy as np
import ml_dtypes

import concourse.bass as bass
import concourse.tile as tile
from concourse import bacc, mybir
from concourse.bass_utils import run_bass_kernel_spmd

B = 4
S = 2048
D = 1024
H = 16
DK = 64
THETA = 10000.0

NEP = H // 2          # head pairs (128-partition groups)
QT = 256              # query tile width (free dim of score matmuls)
KB = 128              # key block (partition dim of score output)
NSLOT = 4             # query slots per core
CNT = [16, 12, 8, 4]  # k-blocks per slot (uniform across cores)
TILES_J = [[7, 5, 3, 1], [6, 4, 2, 0]]  # 256-row q-tile indices per half
VW = DK + 1           # V columns per head incl. trailing ones column

F32R = mybir.dt.float32r
F32 = mybir.dt.float32
BF16 = mybir.dt.bfloat16

_cache = {}


def _build_program():
    if "nc" in _cache:
        return _cache["nc"]

    nc = bacc.Bacc("TRN2")

    xt_d = nc.dram_tensor("xt", [D, S], F32R, kind="ExternalInput")
    xq_d = nc.dram_tensor("xq", [D, NSLOT * QT], F32R, kind="ExternalInput")
    wkt_d = nc.dram_tensor("wkt", [D, D], F32R, kind="ExternalInput")
    wvt_d = nc.dram_tensor("wvt", [D, D], F32R, kind="ExternalInput")
    wqt_d = nc.dram_tensor("wqt", [D, D], F32R, kind="ExternalInput")
    wot_d = nc.dram_tensor("wot", [D, D], F32R, kind="ExternalInput")
    cosk_d = nc.dram_tensor("cosk", [128, S], F32R, kind="ExternalInput")
    sink_d = nc.dram_tensor("sink", [128, S], F32R, kind="ExternalInput")
    cosq_d = nc.dram_tensor("cosq", [128, NSLOT * QT], F32R, kind="ExternalInput")
    sinq_d = nc.dram_tensor("sinq", [128, NSLOT * QT], F32R, kind="ExternalInput")
    mask_d = nc.dram_tensor("mask", [128, NSLOT, 4, QT], BF16, kind="ExternalInput")
    permt_d = nc.dram_tensor("permt", [128, 128], F32R, kind="ExternalInput")
    ones_d = nc.dram_tensor("ones65", [VW, DK], F32R, kind="ExternalInput")
    y_d = nc.dram_tensor("y", [NSLOT * QT, D], F32, kind="ExternalOutput")

    def r(ap):
        return ap

    xt_t = xt_d.rearrange("(n p) s -> p n s", p=128)
    xq_t = xq_d.rearrange("(n p) s -> p n s", p=128)
    wkt_t = wkt_d.rearrange("(n p) e -> p n e", p=128)
    wqt_t = wqt_d.rearrange("(n p) e -> p n e", p=128)

    with tile.TileContext(nc) as tc:
        with (
            tc.tile_pool(name="kv", bufs=1) as kv,
            tc.tile_pool(name="const", bufs=1) as cpool,
        ):
            permt = cpool.tile([128, 128], F32R)
            nc.sync.dma_start(permt[:], permt_d[:])
            ones65 = cpool.tile([VW, DK], F32R)
            nc.sync.dma_start(ones65[:], ones_d[:])

            krot = [kv.tile([128, S], F32R, tag=f"krot{ep}", name=f"krot{ep}")
                    for ep in range(NEP)]
            vt = [kv.tile([128, H * VW], BF16, tag=f"vt{kb}", name=f"vt{kb}")
                  for kb in range(S // KB)]

            # ---------- Phase 1a: K^T projection + RoPE ----------
            with (
                tc.tile_pool(name="wk", bufs=1) as wkp,
                tc.tile_pool(name="ck", bufs=1) as ckp,
                tc.tile_pool(name="xs", bufs=3) as xsp,
                tc.tile_pool(name="t1a", bufs=2) as t1a,
                tc.tile_pool(name="ps1a", bufs=4, space="PSUM") as ps1a,
                tc.tile_pool(name="pp1a", bufs=2, space="PSUM") as pp1a,
            ):
                # K weights as per-head-pair column chunks: first matmul only
                # waits on one 0.5MB chunk instead of the full 4MB
                wk = [wkp.tile([128, 8, 128], F32R, tag=f"wk{ep}", name=f"wk{ep}")
                      for ep in range(NEP)]
                nc.sync.dma_start(wk[0][:], wkt_t[:, :, 0:128])
                xs_first = xsp.tile([128, 8, 512], F32R, tag="xs", name="xs_first")
                nc.gpsimd.dma_start(xs_first[:], xt_t[:, :, 0:512])
                cosk = ckp.tile([128, S], F32R)
                sink = ckp.tile([128, S], F32R)
                nc.sync.dma_start(cosk[:], cosk_d[:])
                nc.sync.dma_start(sink[:], sink_d[:])
                for ep in range(1, NEP):
                    nc.sync.dma_start(wk[ep][:], wkt_t[:, :, ep * 128:(ep + 1) * 128])
                def k_rope(kraw, ep, csl):
                    pp = pp1a.tile([128, 512], F32, tag="perm")
                    nc.tensor.matmul(pp[:], r(permt[:]), r(kraw[:]),
                                     start=True, stop=True)
                    t_c = t1a.tile([128, 512], F32R, tag="t_c")
                    nc.vector.tensor_mul(t_c[:], kraw[:], cosk[:, csl])
                    t_s = t1a.tile([128, 512], F32R, tag="t_s")
                    nc.vector.tensor_mul(t_s[:], pp[:], sink[:, csl])
                    nc.vector.tensor_add(krot[ep][:, csl], t_c[:], t_s[:])

                pend = []
                for st in range(S // 512):
                    if st == 0:
                        xs = xs_first
                    else:
                        xs = xsp.tile([128, 8, 512], F32R, tag="xs")
                        nc.gpsimd.dma_start(xs[:], xt_t[:, :, st * 512:(st + 1) * 512])
                    for ep in range(NEP):
                        pk = ps1a.tile([128, 512], F32, tag="proj")
                        for d in range(8):
                            nc.tensor.matmul(
                                pk[:], r(wk[ep][:, d, :]),
                                r(xs[:, d, :]), start=(d == 0), stop=(d == 7),
                            )
                        kraw = t1a.tile([128, 512], F32R, tag="kraw")
                        nc.any.tensor_copy(kraw[:], pk[:])
                        pend.append((kraw, ep, slice(st * 512, (st + 1) * 512)))
                        if len(pend) > 2:
                            k_rope(*pend.pop(0))
                for p_ in pend:
                    k_rope(*p_)

            # ---------- Phase 1b: V projection (+ones col) ----------
            with (
                tc.tile_pool(name="wv", bufs=1) as wvp,
                tc.tile_pool(name="xs2", bufs=3) as xsp2,
                tc.tile_pool(name="ps1b", bufs=4, space="PSUM") as ps1b,
            ):
                wvt = [wvp.tile([128, D], F32R, tag=f"wv{d}", name=f"wv{d}")
                       for d in range(8)]
                for d in range(8):
                    nc.sync.dma_start(wvt[d][:], wvt_d[d * 128:(d + 1) * 128, :])
                for kb in range(S // KB):
                    nc.vector.memset(
                        vt[kb].rearrange("p (h w) -> p h w", w=VW)[:, :, DK], 1.0
                    )
                for st in range(S // 512):
                    xs2 = xsp2.tile([128, 8, 512], F32R, tag="xs2")
                    nc.gpsimd.dma_start(xs2[:], xt_t[:, :, st * 512:(st + 1) * 512])
                    for half in range(4):
                        kb = 4 * st + half
                        off = half * KB
                        for et in range(2):
                            pv = ps1b.tile([128, 512], F32, tag="vproj")
                            for d in range(8):
                                nc.tensor.matmul(
                                    pv[:], r(xs2[:, d, off:off + KB]),
                                    r(wvt[d][:, et * 512:(et + 1) * 512]),
                                    start=(d == 0), stop=(d == 7),
                                )
                            dst = vt[kb].rearrange("p (h w) -> p h w", w=VW)
                            nc.any.tensor_copy(
                                dst[:, et * 8:(et + 1) * 8, 0:DK],
                                pv[:].rearrange("p (h w) -> p h w", w=DK),
                            )

            # ---------- Phase 1c..2 ----------
            with (
                tc.tile_pool(name="qp", bufs=1) as qp,
                tc.tile_pool(name="mk", bufs=1) as mkp,
            ):
                qrot = [qp.tile([128, NSLOT * QT], F32R, tag=f"qrot{ep}",
                                name=f"qrot{ep}") for ep in range(NEP)]
                masks = mkp.tile([128, NSLOT, 4, QT], BF16)
                nc.sync.dma_start(masks[:], mask_d[:])

                # ---------- Phase 1c: Q^T projection + RoPE ----------
                with (
                    tc.tile_pool(name="wq", bufs=1) as wqp,
                    tc.tile_pool(name="cq", bufs=1) as cqp,
                    tc.tile_pool(name="xs3", bufs=2) as xsp3,
                    tc.tile_pool(name="t1c", bufs=2) as t1c,
                    tc.tile_pool(name="ps1c", bufs=4, space="PSUM") as ps1c,
                    tc.tile_pool(name="pp1c", bufs=2, space="PSUM") as pp1c,
                ):
                    wq = [wqp.tile([128, 8, 128], F32R, tag=f"wq{ep}",
                                   name=f"wq{ep}") for ep in range(NEP)]
                    nc.sync.dma_start(wq[0][:], wqt_t[:, :, 0:128])
                    cosq = cqp.tile([128, NSLOT * QT], F32R)
                    sinq = cqp.tile([128, NSLOT * QT], F32R)
                    nc.sync.dma_start(cosq[:], cosq_d[:])
                    nc.sync.dma_start(sinq[:], sinq_d[:])
                    for ep in range(1, NEP):
                        nc.sync.dma_start(wq[ep][:],
                                          wqt_t[:, :, ep * 128:(ep + 1) * 128])

                    def q_rope(qraw, ep, csl):
                        pp = pp1c.tile([128, QT], F32, tag="qperm")
                        nc.tensor.matmul(pp[:], r(permt[:]), r(qraw[:]),
                                         start=True, stop=True)
                        t_c = t1c.tile([128, QT], F32R, tag="qt_c")
                        nc.vector.tensor_mul(t_c[:], qraw[:], cosq[:, csl])
                        t_s = t1c.tile([128, QT], F32R, tag="qt_s")
                        nc.vector.tensor_mul(t_s[:], pp[:], sinq[:, csl])
                        nc.vector.tensor_add(qrot[ep][:, csl], t_c[:], t_s[:])

                    pend = []
                    for qc in range(NSLOT):
                        xs3 = xsp3.tile([128, 8, QT], F32R, tag="xs3")
                        nc.gpsimd.dma_start(
                            xs3[:], xq_t[:, :, qc * QT:(qc + 1) * QT])
                        for ep in range(NEP):
                            pq = ps1c.tile([128, QT], F32, tag="qproj")
                            for d in range(8):
                                nc.tensor.matmul(
                                    pq[:], r(wq[ep][:, d, :]), r(xs3[:, d, :]),
                                    start=(d == 0), stop=(d == 7),
                                )
                            qraw = t1c.tile([128, QT], F32R, tag="qraw")
                            nc.any.tensor_copy(qraw[:], pq[:])
                            pend.append((qraw, ep, slice(qc * QT, (qc + 1) * QT)))
                            if len(pend) > 2:
                                q_rope(*pend.pop(0))
                    for p_ in pend:
                        q_rope(*p_)

                # ---------- Phase 2: attention + output projection ----------
                with (
                    tc.tile_pool(name="wo", bufs=1) as wop,
                    tc.tile_pool(name="at", bufs=1) as atp,
                    tc.tile_pool(name="ex", bufs=6) as exp_p,
                    tc.tile_pool(name="nrm", bufs=3) as nrmp,
                    tc.tile_pool(name="outs", bufs=3) as outs,
                    tc.tile_pool(name="ps_s", bufs=3, space="PSUM") as ps_s,
                    tc.tile_pool(name="ps_a", bufs=3, space="PSUM") as ps_a,
                    tc.tile_pool(name="ps_b", bufs=1, space="PSUM") as ps_b,
                    tc.tile_pool(name="ps_o", bufs=1, space="PSUM") as ps_o,
                ):
                    wot = [wop.tile([128, D], F32R, tag=f"wo{d}", name=f"wo{d}")
                           for d in range(8)]
                    for d in range(8):
                        nc.sync.dma_start(wot[d][:],
                                          wot_d[d * 128:(d + 1) * 128, :])

                    def normalize(accp, aT, ep):
                        lrow = nrmp.tile([VW, 2, QT], F32R, tag="lrow")
                        with nc.allow_low_precision(
                            reason="f32r tile holds full f32 bits"
                        ):
                            nc.vector.reciprocal(
                                lrow[DK:VW, :, :], accp[DK:VW, :, :]
                            )
                        pb_t = ps_b.tile([DK, 2, QT], F32, tag="bc")
                        nc.tensor.matmul(
                            pb_t[:], r(ones65[DK:VW, :]), r(lrow[DK:VW, :, :]),
                            start=True, stop=True,
                        )
                        rb = nrmp.tile([DK, 2, QT], F32, tag="rb")
                        nc.vector.tensor_copy(rb[:], pb_t[:])
                        nc.vector.tensor_mul(
                            aT[0:DK, :], accp[0:DK, 0, :], rb[:, 0, :]
                        )
                        tmp = nrmp.tile([DK, QT], F32R, tag="nt")
                        nc.vector.tensor_mul(
                            tmp[:], accp[0:DK, 1, :], rb[:, 1, :]
                        )
                        nc.gpsimd.dma_start(aT[DK:128, :], tmp[:])

                    pend_norm = None
                    for sl in range(NSLOT):
                        C = CNT[sl]
                        qsl = slice(sl * QT, (sl + 1) * QT)
                        aT = [atp.tile([128, QT], F32R, tag=f"aT{ep}",
                                       name=f"aT{ep}_{sl}")
                              for ep in range(NEP)]
                        for ep in range(NEP):
                            acc = [ps_a.tile([VW, QT], F32, tag="acc",
                                             name="acc") for _ in range(2)]
                            pend_ex = None

                            def flush_av(kb, exs):
                                for h in range(2):
                                    hh = 2 * ep + h
                                    nc.tensor.matmul(
                                        acc[h][:],
                                        vt[kb][:, hh * VW:(hh + 1) * VW],
                                        exs[h][:],
                                        start=(kb == 0), stop=(kb == C - 1),
                                    )

                            for kb in range(C):
                                exs = []
                                for h in range(2):
                                    pb = h * DK
                                    psc = ps_s.tile([128, QT], F32, tag="sc")
                                    nc.tensor.matmul(
                                        psc[:],
                                        r(krot[ep][pb:pb + DK, kb * KB:(kb + 1) * KB]),
                                        r(qrot[ep][pb:pb + DK, qsl]),
                                        start=True, stop=True,
                                        tile_position=(pb, 0),
                                    )
                                    e = exp_p.tile([128, QT], BF16, tag="ex")
                                    nc.scalar.activation(
                                        e[:], psc[:],
                                        mybir.ActivationFunctionType.Exp,
                                        scale=1.0 / math.sqrt(DK),
                                    )
                                    if kb >= C - 4:
                                        em = exp_p.tile([128, QT], BF16, tag="exm")
                                        nc.vector.tensor_mul(
                                            em[:], e[:], masks[:, sl, kb - (C - 4), :]
                                        )
                                        e = em
                                    exs.append(e)
                                if pend_ex is not None:
                                    flush_av(kb - 1, pend_ex)
                                pend_ex = exs
                            flush_av(C - 1, pend_ex)

                            for h in range(2):
                                lrow = nrmp.tile([VW, QT], F32R, tag="lrow")
                                with nc.allow_low_precision(
                                    reason="f32r holds full f32 bits"
                                ):
                                    nc.vector.reciprocal(
                                        lrow[DK:VW, :], acc[h][DK:VW, :]
                                    )
                                pb_t = ps_b.tile([DK, QT], F32, tag="bc")
                                nc.tensor.matmul(
                                    pb_t[:], r(ones65[DK:VW, :]), r(lrow[DK:VW, :]),
                                    start=True, stop=True,
                                )
                                rb = nrmp.tile([DK, QT], F32, tag="rb")
                                nc.vector.tensor_copy(rb[:], pb_t[:])
                                if h == 0:
                                    nc.vector.tensor_mul(
                                        aT[ep][0:DK, :], acc[h][0:DK, :], rb[:]
                                    )
                                else:
                                    tmp = nrmp.tile([DK, QT], F32R, tag="nt")
                                    nc.vector.tensor_mul(
                                        tmp[:], acc[h][0:DK, :], rb[:]
                                    )
                                    nc.sync.dma_start(aT[ep][DK:128, :], tmp[:])

                        for qs in range(2):
                            for et in range(2):
                                po = ps_o.tile([128, 512], F32, tag="out")
                                for d in range(8):
                                    nc.tensor.matmul(
                                        po[:],
                                        r(aT[d][:, qs * 128:(qs + 1) * 128]),
                                        r(wot[d][:, et * 512:(et + 1) * 512]),
                                        start=(d == 0), stop=(d == 7),
                                    )
                                ot = outs.tile([128, 512], F32, tag="ot")
                                nc.vector.tensor_copy(ot[:], po[:])
                                nc.sync.dma_start(
                                    y_d[sl * QT + qs * 128:
                                        sl * QT + (qs + 1) * 128,
                                        et * 512:(et + 1) * 512],
                                    ot[:],
                                )

    nc.compile()
    nc.finalize()
    _cache["nc"] = nc
    return nc


def _rope_tables(pos):
    """cos/sin tables in [128, n] head-pair layout (row e -> pair (e%64)//2)."""
    k = np.arange(DK // 2, dtype=np.float32)
    inv_freq = (THETA ** (-2.0 * k / DK)).astype(np.float32)
    ang = inv_freq[:, None] * pos.astype(np.float32)[None, :]  # [32, n]
    cos64 = np.repeat(np.cos(ang), 2, axis=0)
    sin64 = np.repeat(np.sin(ang), 2, axis=0)
    return (np.ascontiguousarray(np.concatenate([cos64, cos64], axis=0)),
            np.ascontiguousarray(np.concatenate([sin64, sin64], axis=0)))


def _masks(j):
    """[128, NSLOT, 4, QT] bf16 multiplicative causal masks for half j."""
    p = np.arange(KB)[:, None]
    f = np.arange(QT)[None, :]
    triA = (f >= p).astype(np.float32)
    triB = (f >= p + KB).astype(np.float32)
    ones = np.ones((KB, QT), np.float32)
    zeros = np.zeros((KB, QT), np.float32)
    per_slot = [ones, ones, triA, triB] if j == 0 else [triA, triB, zeros, zeros]
    m = np.stack([np.stack(per_slot, axis=0)] * NSLOT, axis=0)  # [slot, 4, p, f]
    return np.ascontiguousarray(
        m.transpose(2, 0, 1, 3)).astype(ml_dtypes.bfloat16)


def _host_inputs(in_features, token_positions, Wq, Wk, Wv, Wo):
    X = np.asarray(in_features, dtype=np.float32)
    pos = np.asarray(token_positions)
    wqt = np.ascontiguousarray(np.asarray(Wq, np.float32).T)
    wkt = np.ascontiguousarray(np.asarray(Wk, np.float32).T)
    wvt = np.ascontiguousarray(np.asarray(Wv, np.float32).T)
    wot = np.ascontiguousarray(np.asarray(Wo, np.float32).T)
    cosk, sink = _rope_tables(pos)

    permt = np.zeros((128, 128), np.float32)
    for i in range(64):
        permt[2 * i + 1, 2 * i] = -1.0
        permt[2 * i, 2 * i + 1] = 1.0

    in_maps = []
    for core in range(8):
        b, j = core // 2, core % 2
        rows = np.concatenate(
            [np.arange(t * QT, (t + 1) * QT) for t in TILES_J[j]])
        cosq, sinq = _rope_tables(pos[rows])
        in_maps.append({
            "xt": np.ascontiguousarray(X[b].T),
            "xq": np.ascontiguousarray(X[b][rows].T),
            "wkt": wkt, "wvt": wvt, "wqt": wqt, "wot": wot,
            "cosk": cosk, "sink": sink, "cosq": cosq, "sinq": sinq,
            "mask": _masks(j), "permt": permt,
            "ones65": np.ones((VW, DK), np.float32),
        })
    return in_maps


def kernel(in_features, token_positions, Wq, Wk, Wv, Wo):
    nc = _build_program()
    in_maps = _host_inputs(in_features, token_positions, Wq, Wk, Wv, Wo)

    trace = bool(int(os.environ.get("KERNEL_TRACE", "0")))
    res = run_bass_kernel_spmd(nc, in_maps, core_ids=list(range(8)), trace=trace)
    kernel.last_result = res

    out = np.empty((B, S, D), np.float32)
    for core in range(8):
        b, j = core // 2, core % 2
        y = res.results[core]["y"]
        for s_i, t in enumerate(TILES_J[j]):
            out[b, t * QT:(t + 1) * QT, :] = y[s_i * QT:(s_i + 1) * QT, :]
    return out



# revision 3
# speedup vs baseline: 1.3231x; 1.3231x over previous
"""Causal multi-head self-attention with RoPE on 8 Trainium2 NeuronCores.

Sharding: batch (4) x head-group (2) -> 8 cores (tensor parallel over heads).
Each core projects K/V/Q for its 8 heads over the full sequence, runs causal
attention for all 2048 queries, and row-shards the output projection; the
two partial [2048, 1024] outputs per batch are summed on the host (the
all-reduce of the TP out-projection).

Per-core structure (f16 data paths, f32 PSUM):
  phase 1 (per 512-seq chunk st): K^T/Q^T projection (+RoPE via a
    pair-permutation matmul), V projection in natural [seq, feat] layout.
  phase 2 (per 512-query tile T, pipelined with phase 1 of chunk T+1):
    scores S^T[keys, q] per (head, key-block), exp on ACT in [128, 1024]
    bites, AV^T accumulation with a trailing ones-column for the softmax
    denominator, normalize via reciprocal + PE row-broadcast.
  phase 3: output projection of all tiles + DMA out.

PSUM budget (8 banks): proj/perm pool 2, scores pool 2x2 (shared with the
normalize broadcast and the deferred out-projection), AV accumulators 2.
"""

import os
import sys
import math

if "/opt/trn_rl_repo" not in sys.path:
    sys.path.append("/opt/trn_rl_repo")

import numpy as np

import concourse.bass as bass
import concourse.tile as tile
from concourse import bacc, mybir
from concourse.bass_utils import run_bass_kernel_spmd

B = 4
S = 2048
D = 1024
H = 16          # total heads
HC = 8          # heads per core
NEP = HC // 2   # head-pairs per core (128-partition groups)
DK = 64
QT = 512        # query tile
NT = S // QT    # 4 query tiles
ST = 512        # seq chunk for projections
THETA = 10000.0

F32R = mybir.dt.float32r
F32 = mybir.dt.float32
F16 = mybir.dt.float16

_cache = {}


def _build_program():
    if "nc" in _cache:
        return _cache["nc"]

    nc = bacc.Bacc("TRN2")

    xt_d = nc.dram_tensor("xt", [D, S], F16, kind="ExternalInput")
    wkt_d = nc.dram_tensor("wkt", [D, 512], F16, kind="ExternalInput")
    wqt_d = nc.dram_tensor("wqt", [D, 512], F16, kind="ExternalInput")
    wvt_d = nc.dram_tensor("wvt", [D, 512], F16, kind="ExternalInput")
    wot_d = nc.dram_tensor("wot", [512, D], F16, kind="ExternalInput")
    cos_d = nc.dram_tensor("cos", [128, S], F16, kind="ExternalInput")
    sin_d = nc.dram_tensor("sin", [128, S], F16, kind="ExternalInput")
    maska_d = nc.dram_tensor("maska", [128, 2, QT], F16, kind="ExternalInput")
    maskb_d = nc.dram_tensor("maskb", [128, 2, QT], F16, kind="ExternalInput")
    permt_d = nc.dram_tensor("permt", [128, 128], F16, kind="ExternalInput")
    ones_d = nc.dram_tensor("ones1", [128, DK], F32R, kind="ExternalInput")
    y_d = nc.dram_tensor("y", [S, D], F32, kind="ExternalOutput")

    xt_t = xt_d.rearrange("(n p) s -> p n s", p=128)
    wkt_t = wkt_d.rearrange("(n p) e -> p n e", p=128)
    wqt_t = wqt_d.rearrange("(n p) e -> p n e", p=128)
    wvt_t = wvt_d.rearrange("(n p) e -> p n e", p=128)
    wot_t = wot_d.rearrange("(n p) e -> p n e", p=128)

    with tile.TileContext(nc) as tc:
        with (
            tc.tile_pool(name="const", bufs=1) as cpool,
            tc.tile_pool(name="wpool", bufs=1) as wpool,
            tc.tile_pool(name="kv", bufs=1) as kv,
            tc.tile_pool(name="xs", bufs=2) as xsp,
            tc.tile_pool(name="work", bufs=3) as wk_p,
            tc.tile_pool(name="rope", bufs=4) as rope_p,
            tc.tile_pool(name="ex", bufs=4) as exp_p,
            tc.tile_pool(name="nrm", bufs=4) as nrm_p,
            tc.tile_pool(name="at", bufs=1) as at_p,
            tc.tile_pool(name="outs", bufs=3) as outs_p,
            tc.tile_pool(name="ps1", bufs=2, space="PSUM") as ps1,
            tc.tile_pool(name="psc", bufs=2, space="PSUM") as pscp,
            tc.tile_pool(name="pacc", bufs=1, space="PSUM") as paccp,
        ):
            # ---------------- constants / weights ----------------
            permt = cpool.tile([128, 128], F16)
            nc.sync.dma_start(permt[:], permt_d[:])
            ones1 = cpool.tile([128, DK], F32R)
            nc.sync.dma_start(ones1[:], ones_d[:])
            maska = cpool.tile([128, 2, QT], F16)
            nc.sync.dma_start(maska[:], maska_d[:])
            maskb = cpool.tile([128, 2, QT], F16)
            nc.sync.dma_start(maskb[:], maskb_d[:])

            wk = [wpool.tile([128, 8, 128], F16, tag=f"wk{e}", name=f"wk{e}")
                  for e in range(NEP)]
            wq = [wpool.tile([128, 8, 128], F16, tag=f"wq{e}", name=f"wq{e}")
                  for e in range(NEP)]
            for e in range(NEP):
                nc.sync.dma_start(wk[e][:], wkt_t[:, :, e * 128:(e + 1) * 128])
            wv = wpool.tile([128, 8, 512], F16)
            nc.sync.dma_start(wv[:], wvt_t[:, :, :])
            for e in range(NEP):
                nc.sync.dma_start(wq[e][:], wqt_t[:, :, e * 128:(e + 1) * 128])
            wot = [wpool.tile([128, D], F16, tag=f"wo{e}", name=f"wo{e}")
                   for e in range(NEP)]
            for e in range(NEP):
                nc.sync.dma_start(wot[e][:], wot_t[:, e, :])

            cosk = cpool.tile([128, S], F16)
            sink = cpool.tile([128, S], F16)
            nc.sync.dma_start(cosk[:], cos_d[:])
            nc.sync.dma_start(sink[:], sin_d[:])

            # ---------------- persistent activations ----------------
            krot = [kv.tile([128, S], F16, tag=f"krot{e}", name=f"krot{e}")
                    for e in range(NEP)]
            qrot = [kv.tile([128, S], F16, tag=f"qrot{e}", name=f"qrot{e}")
                    for e in range(NEP)]
            vt = [kv.tile([128, HC, DK + 1], F16, tag=f"vt{kb}",
                          name=f"vt{kb}") for kb in range(S // 128)]
            for kb in range(S // 128):
                nc.vector.memset(vt[kb][:, :, DK], 1.0)
            aT = [at_p.tile([128, QT], F16, tag=f"aT{t}_{e}",
                            name=f"aT{t}_{e}")
                  for t in range(NT) for e in range(NEP)]

            def proj_chunk(st):
                """K/Q/V projection + RoPE for seq columns [512 st, 512 st+512)."""
                xs = xsp.tile([128, 8, ST], F16, tag="xs")
                nc.sync.dma_start(xs[:], xt_t[:, :, st * ST:(st + 1) * ST])
                csl = slice(st * ST, (st + 1) * ST)

                for e in range(NEP):
                    for w, rot in ((wk[e], krot[e]), (wq[e], qrot[e])):
                        pk = ps1.tile([128, ST], F32, tag="ps1")
                        for d in range(8):
                            nc.tensor.matmul(pk[:], w[:, d, :], xs[:, d, :],
                                             start=(d == 0), stop=(d == 7))
                        kbf = wk_p.tile([128, ST], F16, tag="kbf")
                        nc.vector.tensor_copy(kbf[:], pk[:])
                        pp = ps1.tile([128, ST], F32, tag="ps1", name="pp")
                        nc.tensor.matmul(pp[:], permt[:], kbf[:],
                                         start=True, stop=True)
                        t_c = rope_p.tile([128, ST], F16, tag="t_c")
                        nc.vector.tensor_mul(t_c[:], kbf[:], cosk[:, csl])
                        t_s = rope_p.tile([128, ST], F16, tag="t_s")
                        nc.vector.tensor_mul(t_s[:], pp[:], sink[:, csl])
                        nc.vector.tensor_add(rot[:, csl], t_c[:], t_s[:])

                # V projection, natural [seq, feat] layout
                for half in range(ST // 128):
                    pv = ps1.tile([128, 512], F32, tag="ps1", name="pv")
                    off = half * 128
                    for d in range(8):
                        nc.tensor.matmul(pv[:], xs[:, d, off:off + 128],
                                         wv[:, d, :], start=(d == 0),
                                         stop=(d == 7))
                    kb = st * (ST // 128) + half
                    dst = vt[kb][:, :, 0:DK]
                    nc.vector.tensor_copy(
                        dst, pv[:].rearrange("p (h w) -> p h w", w=DK))

            def attn_tile(t):
                """Attention for queries [512 t, 512 t + 512)."""
                qsl = slice(t * QT, (t + 1) * QT)
                nkbp = 2 * t + 2   # key-block pairs (256 keys each)
                for e in range(NEP):
                    for h in range(2):
                        hh = 2 * e + h
                        pb = h * DK
                        acc = paccp.tile([DK + 1, QT], F32, tag=f"acc{h}",
                                         name=f"acc{h}")
                        for c in range(nkbp):
                            psc = pscp.tile([128, 2, QT], F32, tag="psc")
                            for j in range(2):
                                kb = 2 * c + j
                                nc.tensor.matmul(
                                    psc[:, j, :],
                                    krot[e][pb:pb + DK, kb * 128:(kb + 1) * 128],
                                    qrot[e][pb:pb + DK, qsl],
                                    start=True, stop=True,
                                    tile_position=(pb, 0),
                                )
                            ex = exp_p.tile([128, 2, QT], F16, tag="ex")
                            nc.scalar.activation(
                                ex[:], psc[:],
                                mybir.ActivationFunctionType.Exp,
                                scale=1.0 / math.sqrt(DK),
                            )
                            if c >= nkbp - 2:
                                msk = maska if c == nkbp - 2 else maskb
                                exm = exp_p.tile([128, 2, QT], F16, tag="exm")
                                nc.vector.tensor_mul(exm[:], ex[:], msk[:])
                                ex = exm
                            for j in range(2):
                                kb = 2 * c + j
                                nc.tensor.matmul(
                                    acc[:], vt[kb][:, hh, :], ex[:, j, :],
                                    start=(c == 0 and j == 0),
                                    stop=(c == nkbp - 1 and j == 1),
                                )
                        # normalize: denominator on partition DK
                        lrow = nrm_p.tile([DK + 1, QT], F32R, tag="lrow")
                        with nc.allow_low_precision(
                            reason="f32r tile holds full f32 bits"
                        ):
                            nc.vector.reciprocal(lrow[DK:DK + 1, :],
                                                 acc[DK:DK + 1, :])
                        accsb = nrm_p.tile([DK, QT], F16, tag="accsb")
                        nc.scalar.copy(accsb[:], acc[0:DK, :])
                        pbt = pscp.tile([DK, QT], F32, tag="psc", name="pbt")
                        nc.tensor.matmul(pbt[:], ones1[DK:DK + 1, :],
                                         lrow[DK:DK + 1, :],
                                         start=True, stop=True)
                        rb = nrm_p.tile([DK, QT], F16, tag="rb")
                        nc.vector.tensor_copy(rb[:], pbt[:])
                        if h == 0:
                            nc.vector.tensor_mul(aT[t * NEP + e][0:DK, :],
                                                 accsb[:], rb[:])
                        else:
                            tmp = nrm_p.tile([DK, QT], F16, tag="tmp")
                            nc.vector.tensor_mul(tmp[:], accsb[:], rb[:])
                            nc.sync.dma_start(aT[t * NEP + e][DK:128, :],
                                              tmp[:])

            # -------- pipelined schedule --------
            proj_chunk(0)
            proj_chunk(1)
            attn_tile(0)
            proj_chunk(2)
            attn_tile(1)
            proj_chunk(3)
            attn_tile(2)
            attn_tile(3)

            # -------- output projection (partial sums over our heads) ------
            for t in range(NT):
                for qc in range(QT // 128):
                    po = pscp.tile([128, 2, 512], F32, tag="psc", name="po")
                    for et in range(2):
                        for e in range(NEP):
                            nc.tensor.matmul(
                                po[:, et, :],
                                aT[t * NEP + e][:, qc * 128:(qc + 1) * 128],
                                wot[e][:, et * 512:(et + 1) * 512],
                                start=(e == 0), stop=(e == NEP - 1),
                            )
                    ot = outs_p.tile([128, D], F32, tag="ot")
                    nc.vector.tensor_copy(ot[:], po[:].rearrange("p a b -> p (a b)"))
                    nc.sync.dma_start(
                        y_d[t * QT + qc * 128: t * QT + (qc + 1) * 128, :],
                        ot[:])

    nc.compile()
    nc.finalize()
    _cache["nc"] = nc
    return nc


def _rope_tables(pos):
    """cos/sin in [128, S] transposed head-pair layout (row r -> pair (r%64)//2)."""
    k = np.arange(DK // 2, dtype=np.float32)
    inv_freq = (THETA ** (-2.0 * k / DK)).astype(np.float32)
    ang = inv_freq[:, None] * pos.astype(np.float32)[None, :]   # [32, S]
    cos64 = np.repeat(np.cos(ang), 2, axis=0)
    sin64 = np.repeat(np.sin(ang), 2, axis=0)
    return (np.ascontiguousarray(np.concatenate([cos64, cos64], 0)).astype(np.float16),
            np.ascontiguousarray(np.concatenate([sin64, sin64], 0)).astype(np.float16))


def _masks():
    """maska/maskb [128, 2, 512] f16: diagonal key-block-pair masks."""
    p = np.arange(128)[:, None]
    f = np.arange(QT)[None, :]
    tris = [(f >= p + 128 * j).astype(np.float16) for j in range(4)]
    maska = np.stack([tris[0], tris[1]], axis=1)
    maskb = np.stack([tris[2], tris[3]], axis=1)
    return np.ascontiguousarray(maska), np.ascontiguousarray(maskb)


def _host_inputs(in_features, token_positions, Wq, Wk, Wv, Wo):
    X = np.asarray(in_features, dtype=np.float32)
    pos = np.asarray(token_positions)
    cos, sin = _rope_tables(pos)
    maska, maskb = _masks()

    permt = np.zeros((128, 128), np.float16)
    for i in range(64):
        permt[2 * i + 1, 2 * i] = -1.0
        permt[2 * i, 2 * i + 1] = 1.0

    Wqf = np.asarray(Wq, np.float32)
    Wkf = np.asarray(Wk, np.float32)
    Wvf = np.asarray(Wv, np.float32)
    Wof = np.asarray(Wo, np.float32)

    in_maps = []
    for core in range(8):
        b, hg = core // 2, core % 2
        rows = slice(hg * 512, (hg + 1) * 512)
        in_maps.append({
            "xt": np.ascontiguousarray(X[b].T).astype(np.float16),
            "wkt": np.ascontiguousarray(Wkf[rows, :].T).astype(np.float16),
            "wqt": np.ascontiguousarray(Wqf[rows, :].T).astype(np.float16),
            "wvt": np.ascontiguousarray(Wvf[rows, :].T).astype(np.float16),
            "wot": np.ascontiguousarray(Wof[:, rows].T).astype(np.float16),
            "cos": cos, "sin": sin,
            "maska": maska, "maskb": maskb,
            "permt": permt,
            "ones1": np.ones((128, DK), np.float32),
        })
    return in_maps


def kernel(in_features, token_positions, Wq, Wk, Wv, Wo):
    nc = _build_program()
    in_maps = _host_inputs(in_features, token_positions, Wq, Wk, Wv, Wo)

    trace = bool(int(os.environ.get("KERNEL_TRACE", "0")))
    res = run_bass_kernel_spmd(nc, in_maps, core_ids=list(range(8)), trace=trace)
    kernel.last_result = res

    out = np.empty((B, S, D), np.float32)
    for b in range(B):
        out[b] = res.results[2 * b]["y"] + res.results[2 * b + 1]["y"]
    return out


# revision 8
# speedup vs baseline: 1.5776x; 1.1923x over previous
"""Causal multi-head self-attention with RoPE on 8 Trainium2 NeuronCores.

Sharding: batch (4) x head-group (2) -> 8 cores (tensor parallel over heads).
Each core projects K/V/Q for its 8 heads over the full sequence, runs causal
attention for all 2048 queries, and row-shards the output projection; the
two partial [2048, 1024] outputs per batch are summed on the host (the
all-reduce of the TP out-projection).

Per-core structure (f16 data paths, f32 PSUM):
  phase 1 (per 512-seq chunk st): K^T/Q^T projection (+RoPE via a
    pair-permutation matmul), V projection in natural [seq, feat] layout.
  phase 2 (per 512-query tile T, pipelined with phase 1 of chunk T+1):
    scores S^T[keys, q] per (head, key-block), exp on ACT in [128, 1024]
    bites, AV^T accumulation with a trailing ones-column for the softmax
    denominator, normalize via reciprocal + PE row-broadcast.
  phase 3: output projection of all tiles + DMA out.

PSUM budget (8 banks): proj/perm pool 2, scores pool 2x2 (shared with the
normalize broadcast and the deferred out-projection), AV accumulators 2.
"""

import os
import sys
import math

if "/opt/trn_rl_repo" not in sys.path:
    sys.path.append("/opt/trn_rl_repo")

import numpy as np

import concourse.bass as bass
import concourse.tile as tile
from concourse import bacc, mybir
from concourse.bass_utils import run_bass_kernel_spmd

B = 4
S = 2048
D = 1024
H = 16          # total heads
HC = 8          # heads per core
NEP = HC // 2   # head-pairs per core (128-partition groups)
DK = 64
QT = 512        # query tile
NT = S // QT    # 4 query tiles
ST = 512        # seq chunk for projections
THETA = 10000.0

F32R = mybir.dt.float32r
F32 = mybir.dt.float32
F16 = mybir.dt.float16

_cache = {}


def _build_program():
    if "nc" in _cache:
        return _cache["nc"]

    nc = bacc.Bacc("TRN2")

    xt_d = nc.dram_tensor("xt", [D, S], F16, kind="ExternalInput")
    wkt_d = nc.dram_tensor("wkt", [D, 512], F16, kind="ExternalInput")
    wqt_d = nc.dram_tensor("wqt", [D, 512], F16, kind="ExternalInput")
    wvt_d = nc.dram_tensor("wvt", [D, 512], F16, kind="ExternalInput")
    wot_d = nc.dram_tensor("wot", [512, D], F16, kind="ExternalInput")
    cos_d = nc.dram_tensor("cos", [128, S], F16, kind="ExternalInput")
    sin_d = nc.dram_tensor("sin", [128, S], F16, kind="ExternalInput")
    maska_d = nc.dram_tensor("maska", [128, 2, QT], F16, kind="ExternalInput")
    permt_d = nc.dram_tensor("permt", [128, 128], F16, kind="ExternalInput")
    ones_d = nc.dram_tensor("ones1", [128, DK], F32R, kind="ExternalInput")
    y_d = nc.dram_tensor("y", [S, D], F32, kind="ExternalOutput")

    xt_t = xt_d.rearrange("(n p) s -> p n s", p=128)
    wkt_t = wkt_d.rearrange("(n p) e -> p n e", p=128)
    wqt_t = wqt_d.rearrange("(n p) e -> p n e", p=128)
    wvt_t = wvt_d.rearrange("(n p) e -> p n e", p=128)
    wot_t = wot_d.rearrange("(n p) e -> p n e", p=128)

    with tile.TileContext(nc) as tc:
        with (
            tc.tile_pool(name="const", bufs=1) as cpool,
            tc.tile_pool(name="wpool", bufs=1) as wpool,
            tc.tile_pool(name="kv", bufs=1) as kv,
            tc.tile_pool(name="xs", bufs=2) as xsp,
            tc.tile_pool(name="work", bufs=3) as wk_p,
            tc.tile_pool(name="rope", bufs=4) as rope_p,
            tc.tile_pool(name="ex", bufs=4) as exp_p,
            tc.tile_pool(name="nrm", bufs=4) as nrm_p,
            tc.tile_pool(name="at", bufs=1) as at_p,
            tc.tile_pool(name="outs", bufs=3) as outs_p,
            tc.tile_pool(name="ps1", bufs=2, space="PSUM") as ps1,
            tc.tile_pool(name="psc", bufs=2, space="PSUM") as pscp,
            tc.tile_pool(name="pacc", bufs=1, space="PSUM") as paccp,
        ):
            # ---------------- constants / weights ----------------
            # DMA order matters for the cold start: the first K-projection
            # needs xs(st0) [gpsimd queue, issued in proj_chunk] + wk[0] only.
            wk = [wpool.tile([128, 8, 128], F16, tag=f"wk{e}", name=f"wk{e}")
                  for e in range(NEP)]
            wq = [wpool.tile([128, 8, 128], F16, tag=f"wq{e}", name=f"wq{e}")
                  for e in range(NEP)]
            nc.sync.dma_start(wk[0][:], wkt_t[:, :, 0:128])
            permt = cpool.tile([128, 128], F16)
            nc.sync.dma_start(permt[:], permt_d[:])
            cosk = cpool.tile([128, S], F16)
            sink = cpool.tile([128, S], F16)
            nc.sync.dma_start(cosk[:], cos_d[:])
            nc.sync.dma_start(sink[:], sin_d[:])
            nc.sync.dma_start(wq[0][:], wqt_t[:, :, 0:128])
            for e in range(1, NEP):
                nc.sync.dma_start(wk[e][:], wkt_t[:, :, e * 128:(e + 1) * 128])
                nc.sync.dma_start(wq[e][:], wqt_t[:, :, e * 128:(e + 1) * 128])
            wv = wpool.tile([128, 8, 512], F16)
            nc.sync.dma_start(wv[:], wvt_t[:, :, :])
            ones1 = cpool.tile([128, DK], F32R)
            nc.sync.dma_start(ones1[:], ones_d[:])
            maska = cpool.tile([128, 2, QT], F16)
            nc.sync.dma_start(maska[:], maska_d[:])
            wot = [wpool.tile([128, D], F16, tag=f"wo{e}", name=f"wo{e}")
                   for e in range(NEP)]
            for e in range(NEP):
                nc.sync.dma_start(wot[e][:], wot_t[:, e, :])

            # ---------------- persistent activations ----------------
            krot = [kv.tile([128, S], F16, tag=f"krot{e}", name=f"krot{e}")
                    for e in range(NEP)]
            qrot = [kv.tile([128, S], F16, tag=f"qrot{e}", name=f"qrot{e}")
                    for e in range(NEP)]
            vt = [kv.tile([128, HC, DK + 1], F16, tag=f"vt{kb}",
                          name=f"vt{kb}") for kb in range(S // 128)]
            for kb in range(S // 128):
                nc.vector.memset(vt[kb][:, :, DK], 1.0)
            aT = [at_p.tile([128, QT], F16, tag=f"aT{t}_{e}",
                            name=f"aT{t}_{e}")
                  for t in range(NT) for e in range(NEP)]

            def proj_chunk(st):
                """K/Q/V projection + RoPE for seq columns [512 st, 512 st+512)."""
                xs = xsp.tile([128, 8, ST], F16, tag="xs")
                nc.gpsimd.dma_start(xs[:], xt_t[:, :, st * ST:(st + 1) * ST])
                csl = slice(st * ST, (st + 1) * ST)

                for e in range(NEP):
                    for w, rot in ((wk[e], krot[e]), (wq[e], qrot[e])):
                        pk = ps1.tile([128, ST], F32, tag="ps1")
                        for d in range(8):
                            nc.tensor.matmul(pk[:], w[:, d, :], xs[:, d, :],
                                             start=(d == 0), stop=(d == 7))
                        kbf = wk_p.tile([128, ST], F16, tag="kbf")
                        nc.vector.tensor_copy(kbf[:], pk[:])
                        pp = ps1.tile([128, ST], F32, tag="ps1", name="pp")
                        nc.tensor.matmul(pp[:], permt[:], kbf[:],
                                         start=True, stop=True)
                        t_c = rope_p.tile([128, ST], F16, tag="t_c")
                        nc.vector.tensor_mul(t_c[:], kbf[:], cosk[:, csl])
                        t_s = rope_p.tile([128, ST], F16, tag="t_s")
                        nc.vector.tensor_mul(t_s[:], pp[:], sink[:, csl])
                        nc.vector.tensor_add(rot[:, csl], t_c[:], t_s[:])

                # V projection, natural [seq, feat] layout
                for half in range(ST // 128):
                    pv = ps1.tile([128, 512], F32, tag="ps1", name="pv")
                    off = half * 128
                    for d in range(8):
                        nc.tensor.matmul(pv[:], xs[:, d, off:off + 128],
                                         wv[:, d, :], start=(d == 0),
                                         stop=(d == 7))
                    kb = st * (ST // 128) + half
                    dst = vt[kb][:, :, 0:DK]
                    nc.vector.tensor_copy(
                        dst, pv[:].rearrange("p (h w) -> p h w", w=DK))

            pend_norm = [None]

            def normalize(acc, t, e, h):
                """Softmax denominator divide + aT assembly for one (e, h)."""
                lrow = nrm_p.tile([DK + 1, QT], F32R, tag="lrow")
                with nc.allow_low_precision(
                    reason="f32r tile holds full f32 bits"
                ):
                    nc.vector.reciprocal(lrow[DK:DK + 1, :],
                                         acc[DK:DK + 1, :])
                accsb = nrm_p.tile([DK, QT], F16, tag="accsb")
                nc.vector.tensor_copy(accsb[:], acc[0:DK, :])
                pbt = pscp.tile([DK, QT], F32, tag="psc", name="pbt")
                nc.tensor.matmul(pbt[:], ones1[DK:DK + 1, :],
                                 lrow[DK:DK + 1, :], start=True, stop=True)
                rb = nrm_p.tile([DK, QT], F16, tag="rb")
                nc.vector.tensor_copy(rb[:], pbt[:])
                if h == 0:
                    nc.vector.tensor_mul(aT[t * NEP + e][0:DK, :],
                                         accsb[:], rb[:])
                else:
                    tmp = nrm_p.tile([DK, QT], F16, tag="tmp")
                    nc.vector.tensor_mul(tmp[:], accsb[:], rb[:])
                    nc.sync.dma_start(aT[t * NEP + e][DK:128, :], tmp[:])

            def attn_tile(t):
                """Attention for queries [512 t, 512 t + 512)."""
                qsl = slice(t * QT, (t + 1) * QT)
                qsl2 = slice(t * QT + 256, (t + 1) * QT)  # diagonal half
                nkbp = 2 * t + 2   # key-block pairs (256 keys each)
                for e in range(NEP):
                    for h in range(2):
                        hh = 2 * e + h
                        pb = h * DK
                        acc = paccp.tile([DK + 1, QT], F32, tag=f"acc{h}",
                                         name=f"acc{h}")
                        for c in range(nkbp):
                            # last pair: only the upper query half is unmasked
                            half = c == nkbp - 1
                            qw = 256 if half else QT
                            qs = qsl2 if half else qsl
                            psc = pscp.tile([128, 2, qw], F32, tag="psc")
                            for j in range(2):
                                kb = 2 * c + j
                                nc.tensor.matmul(
                                    psc[:, j, :],
                                    krot[e][pb:pb + DK, kb * 128:(kb + 1) * 128],
                                    qrot[e][pb:pb + DK, qs],
                                    start=True, stop=True,
                                    tile_position=(pb, 0),
                                )
                            ex = exp_p.tile([128, 2, qw], F16, tag="ex")
                            nc.scalar.activation(
                                ex[:], psc[:],
                                mybir.ActivationFunctionType.Exp,
                                scale=1.0 / math.sqrt(DK),
                            )
                            if c >= nkbp - 2:
                                msk = maska[:, :, 0:qw]
                                exm = exp_p.tile([128, 2, qw], F16, tag="exm")
                                nc.vector.tensor_mul(exm[:], ex[:], msk)
                                ex = exm
                            dst = acc[:, 256:QT] if half else acc[:]
                            for j in range(2):
                                kb = 2 * c + j
                                nc.tensor.matmul(
                                    dst, vt[kb][:, hh, :], ex[:, j, :],
                                    start=(c == 0 and j == 0),
                                    stop=(c == nkbp - 1 and j == 1),
                                )
                            if c == min(1, nkbp - 1) and pend_norm[0]:
                                normalize(*pend_norm[0])
                                pend_norm[0] = None
                        pend_norm[0] = (acc, t, e, h)

            # -------- pipelined schedule --------
            proj_chunk(0)
            proj_chunk(1)
            attn_tile(0)
            proj_chunk(2)
            attn_tile(1)
            proj_chunk(3)
            attn_tile(2)
            attn_tile(3)
            normalize(*pend_norm[0])
            pend_norm[0] = None

            # -------- output projection (partial sums over our heads) ------
            for t in range(NT):
                for qc in range(QT // 128):
                    po = pscp.tile([128, 2, 512], F32, tag="psc", name="po")
                    for et in range(2):
                        for e in range(NEP):
                            nc.tensor.matmul(
                                po[:, et, :],
                                aT[t * NEP + e][:, qc * 128:(qc + 1) * 128],
                                wot[e][:, et * 512:(et + 1) * 512],
                                start=(e == 0), stop=(e == NEP - 1),
                            )
                    ot = outs_p.tile([128, D], F32, tag="ot")
                    nc.vector.tensor_copy(ot[:], po[:].rearrange("p a b -> p (a b)"))
                    nc.sync.dma_start(
                        y_d[t * QT + qc * 128: t * QT + (qc + 1) * 128, :],
                        ot[:])

    nc.compile()
    nc.finalize()
    _cache["nc"] = nc
    return nc


def _rope_tables(pos):
    """cos/sin in [128, S] transposed head-pair layout (row r -> pair (r%64)//2)."""
    k = np.arange(DK // 2, dtype=np.float32)
    inv_freq = (THETA ** (-2.0 * k / DK)).astype(np.float32)
    ang = inv_freq[:, None] * pos.astype(np.float32)[None, :]   # [32, S]
    cos64 = np.repeat(np.cos(ang), 2, axis=0)
    sin64 = np.repeat(np.sin(ang), 2, axis=0)
    return (np.ascontiguousarray(np.concatenate([cos64, cos64], 0)).astype(np.float16),
            np.ascontiguousarray(np.concatenate([sin64, sin64], 0)).astype(np.float16))


def _masks():
    """maska [128, 2, 512] f16: diagonal key-block-pair masks."""
    p = np.arange(128)[:, None]
    f = np.arange(QT)[None, :]
    tris = [(f >= p + 128 * j).astype(np.float16) for j in range(2)]
    return np.ascontiguousarray(np.stack(tris, axis=1))


def _host_inputs(in_features, token_positions, Wq, Wk, Wv, Wo):
    X = np.asarray(in_features, dtype=np.float32)
    pos = np.asarray(token_positions)
    cos, sin = _rope_tables(pos)
    maska = _masks()

    permt = np.zeros((128, 128), np.float16)
    for i in range(64):
        permt[2 * i + 1, 2 * i] = -1.0
        permt[2 * i, 2 * i + 1] = 1.0

    Wqf = np.asarray(Wq, np.float32)
    Wkf = np.asarray(Wk, np.float32)
    Wvf = np.asarray(Wv, np.float32)
    Wof = np.asarray(Wo, np.float32)

    in_maps = []
    for core in range(8):
        b, hg = core // 2, core % 2
        rows = slice(hg * 512, (hg + 1) * 512)
        in_maps.append({
            "xt": np.ascontiguousarray(X[b].T).astype(np.float16),
            "wkt": np.ascontiguousarray(Wkf[rows, :].T).astype(np.float16),
            "wqt": np.ascontiguousarray(Wqf[rows, :].T).astype(np.float16),
            "wvt": np.ascontiguousarray(Wvf[rows, :].T).astype(np.float16),
            "wot": np.ascontiguousarray(Wof[:, rows].T).astype(np.float16),
            "cos": cos, "sin": sin,
            "maska": maska,
            "permt": permt,
            "ones1": np.ones((128, DK), np.float32),
        })
    return in_maps


def kernel(in_features, token_positions, Wq, Wk, Wv, Wo):
    nc = _build_program()
    in_maps = _host_inputs(in_features, token_positions, Wq, Wk, Wv, Wo)

    trace = bool(int(os.environ.get("KERNEL_TRACE", "0")))
    res = run_bass_kernel_spmd(nc, in_maps, core_ids=list(range(8)), trace=trace)
    kernel.last_result = res

    out = np.empty((B, S, D), np.float32)
    for b in range(B):
        out[b] = res.results[2 * b]["y"] + res.results[2 * b + 1]["y"]
    return out
